# revision 16
# baseline (speedup 1.0000x reference)
"""Sharded cosine-similarity kNN (k=5) + weighted centroid on 8 TRN2 NeuronCores.

Strategy (fp8 + TensorEngine, ~3.8x over the f32 DVE/ACT baseline):
  - Host: normalize every database row to norm 64 (a per-row quantization
    scale), quantize to fp8 e4m3 (TRN variant; |x| <= ~14 << 240), and ship
    each core's 62500-row shard TRANSPOSED as 3 chunk-pairs x [128 dims x
    2 x 62976 rows] (padded with -q_norm rows, cosine -1, never selected).
    HBM traffic per core: 48 MB fp8 vs 192 MB f32 -> ~135 us DMA roofline
    at ~358 GB/s/core.
  - Device: all dot products on the TensorEngine with DoubleRow fp8 (256-deep
    contraction, 2 MACs/cell/cycle). For window i (512 rows), pair cp:
    matmul(lhsT = one-hot weight strip [128, 2, 128] with q-pair in column i,
    rhs = db_t[cp][:, :, window]) accumulates cos*4096 into PSUM partition i.
    The strip is one [128, 2, 256] SBUF slice per pair with q at column 128:
    slicing [128-i : 256-i] puts q at local column i, so one tiny host tensor
    serves all 123 windows. 369 matmuls, N=512, one PSUM bank, one
    accumulation group; DMA in 16-window blocks (16 KB/partition runs),
    triple-buffered; aux DMAs (q strips, outputs) ride the second HWDGE ring
    so the db stream never stalls.
  - Epilogue: copy PSUM->SBUF (ACT), DVE max/max_index -> top-8 rows per
    window, DMA out [128,8] values + column indices.
  - Host: map (core, window, col) -> rows, rescore the ~7.9k candidates
    exactly in f64 from the original f32 database, exact top-5 + weighted
    centroid. fp8 sim noise is ~0.0014 (std) while a true top-5 row leads its
    512-row window by >20 sigma, so candidate recall is exact (validated
    offline and on HW: rel err 1.1e-7).

Measured (R-loop differencing, interleaved): ~147-154 us/iteration vs 563 us
baseline. Plain fp8 (no DoubleRow) is PE-streaming-bound at ~160-195 us;
DoubleRow halves PE streaming and makes the kernel DMA-bound.

Environment workaround: this container's walrus build rejects any instruction
with more than one semaphore wait; see split_sync_waits().
"""

import contextlib

import numpy as np
import ml_dtypes

import concourse.bass as bass
import concourse.mybir as mybir
from concourse.tile import TileContext
from concourse.bass_utils import run_bass_kernel_spmd

N_CORES = 8
D = 768
N_ROWS = 500000
SHARD = N_ROWS // N_CORES   # 62500
P = 128
C = D // P                  # 6 contraction chunks
WIN = 512                   # rows per window (= matmul N = one PSUM bank)
NW = 123                    # windows per core (123*512 = 62976 >= 62500)
PADROWS = NW * WIN          # 62976
SCALE = 64.0                # per-row quantization scale (norm target)
BLOCK_W = 16                # windows per DMA block (16*512 B = 8 KB runs)
K = 5
COS_EPS = 1e-8
W_EPS = 1e-6

_f32 = mybir.dt.float32
_u32 = mybir.dt.uint32
_f8 = mybir.dt.float8e4
_np_f8 = ml_dtypes.float8_e4m3

_wsplit_ctr = [0]


def split_sync_waits(nc):
    """Workaround for this container's walrus build: it rejects any instruction
    carrying more than ONE semaphore wait ("Too many sync wait commands" in
    setupSyncWait during codegen). Tile's scheduler freely attaches several
    waits to one instruction, so after TileContext scheduling we split them:
    every instruction keeps its last wait, and each extra wait is hoisted onto
    its own NoOp placed immediately before it in the same basic block (same
    engine, so program order preserves wait-before-execute semantics)."""
    for f in nc.m.functions:
        for b in f.blocks:
            needs_fix = any(
                getattr(i, "sync_info", None) is not None
                and i.sync_info.on_wait
                and len(i.sync_info.on_wait) > 1
                for i in b.instructions
            )
            if not needs_fix:
                continue
            new_insts = []
            for inst in b.instructions:
                si = getattr(inst, "sync_info", None)
                if si is not None and si.on_wait and len(si.on_wait) > 1:
                    waits = list(si.on_wait)
                    for w in waits[:-1]:
                        _wsplit_ctr[0] += 1
                        nop = mybir.InstNoOp(
                            name=f"WSPLIT-{_wsplit_ctr[0]}", ins=[], outs=[]
                        )
                        nop.engine = inst.engine
                        nop.sync_info = mybir.SyncInfo(on_wait=[w], on_update=[])
                        new_insts.append(nop)
                    inst.sync_info = mybir.SyncInfo(
                        on_wait=[waits[-1]], on_update=list(si.on_update or [])
                    )
                new_insts.append(inst)
            b.instructions[:] = new_insts
    return nc


def _blocks(block_w: int = BLOCK_W):
    """DMA block partition of the 123 windows. Sizes shrink at the end so the
    final PE burst + epilogue tail after the last DMA is minimal."""
    sizes = []
    rem = NW
    while rem > block_w:
        sizes.append(block_w)
        rem -= block_w
    while rem > 2:
        h = (rem + 1) // 2
        sizes.append(h)
        rem -= h
    while rem > 0:
        sizes.append(1)
        rem -= 1
    out = []
    w = 0
    for s in sizes:
        out.append((w, s))
        w += s
    return out


def build_nc(repeat: int = 1, db_bufs: int = 3, double_row: bool = True,
             block_w: int = BLOCK_W, out_vals: bool = True,
             paired: bool = True, ring2: bool = True,
             split_ep: bool = False, dma2: bool = False):
    nc = bass.Bass()
    dbt = nc.dram_tensor("dbt", [C * P * PADROWS], _f8, kind="ExternalInput")
    qs = nc.dram_tensor("qs", [P, C, 256], _f8, kind="ExternalInput")
    n_out = 2 * P if split_ep else P
    outv = (nc.dram_tensor("outv", [n_out, 8], _f32, kind="ExternalOutput")
            if out_vals else None)
    outi = nc.dram_tensor("outi", [n_out, 8], _u32, kind="ExternalOutput")

    blocks = _blocks(block_w)
    kg = 2 if double_row else 1      # chunks consumed per matmul
    pm = mybir.MatmulPerfMode.DoubleRow if double_row else None
    n_mm = NW * C // kg
    halves = [(0, 62), (62, NW)] if split_ep else [(0, NW)]

    with TileContext(nc) as tc:
        with (
            tc.tile_pool(name="persist", bufs=1) as persist,
            tc.tile_pool(name="dbp", bufs=db_bufs) as dbp,
            tc.tile_pool(name="psum", bufs=1, space="PSUM") as psp,
        ):
            loop = tc.For_i(0, repeat, 1) if repeat > 1 else contextlib.nullcontext()
            with loop:
                sims_ps = []
                for h in range(len(halves)):
                    ps_h = psp.tile([P, WIN], _f32, tag=f"sims{h}",
                                    name=f"sims{h}")
                    sims_ps.append(ps_h)

                # q strips: qt[:, c, 128] = q chunk c; lhsT slice
                # [:, c:c+kg, 128-i : 256-i] puts q_c one-hot at local col i.
                qt = persist.tile([P, C, 256], _f8, tag="qt")
                aux = nc.scalar if ring2 else nc.sync
                aux.dma_start(qt[:], qs[:])

                def half_of(i):
                    for h, (a, b) in enumerate(halves):
                        if a <= i < b:
                            return h
                    raise AssertionError

                def epilogue(h):
                    a, b = halves[h]
                    sims = persist.tile([P, WIN], _f32, tag=f"sims_sb{h}")
                    nc.scalar.copy(sims[:], sims_ps[h][:])
                    vals8 = persist.tile([P, 8], _f32, tag=f"vals8{h}")
                    idx8 = persist.tile([P, 8], _u32, tag=f"idx8{h}")
                    nc.vector.max(vals8[:], sims[:])
                    nc.vector.max_index(idx8[:], vals8[:], sims[:])
                    if out_vals:
                        aux.dma_start(outv[h * P : (h + 1) * P, :], vals8[:])
                    aux.dma_start(outi[h * P : (h + 1) * P, :], idx8[:])

                group_n = [(b - a) * C // kg for (a, b) in halves]
                group_mm = [0] * len(halves)
                for bi, (w0, nwb) in enumerate(blocks):
                    deng = nc.scalar if (dma2 and bi % 2) else nc.sync
                    width = nwb * WIN
                    tiles = []
                    for cp in range(C // kg):
                        t = dbp.tile([P, kg, block_w * WIN], _f8, tag=f"db{cp}")
                        if paired:
                            assert kg == 2
                            src = dbt[cp * P * 2 * PADROWS
                                      : (cp + 1) * P * 2 * PADROWS]
                            src = src.rearrange("(p a r) -> p a r", a=2,
                                                r=PADROWS)
                            deng.dma_start(
                                t[:, :, :width],
                                src[:, :, w0 * WIN : w0 * WIN + width],
                            )
                        else:
                            for ko in range(kg):
                                c = cp * kg + ko
                                src = dbt[c * P * PADROWS
                                          : (c + 1) * P * PADROWS]
                                src = src.rearrange("(p r) -> p r", r=PADROWS)
                                deng.dma_start(
                                    t[:, ko : ko + 1, :width],
                                    src[:, w0 * WIN : w0 * WIN + width],
                                )
                        tiles.append(t)
                    for wl in range(nwb):
                        i = w0 + wl
                        h = half_of(i)
                        col = i - halves[h][0]
                        for cp in range(C // kg):
                            nc.tensor.matmul(
                                out=sims_ps[h][:, :],
                                lhsT=qt[:, cp * kg : (cp + 1) * kg,
                                        (P - col) : (2 * P - col)],
                                rhs=tiles[cp][:, :, wl * WIN : (wl + 1) * WIN],
                                start=(group_mm[h] == 0),
                                stop=(group_mm[h] == group_n[h] - 1),
                                perf_mode=pm,
                            )
                            group_mm[h] += 1
                            if group_mm[h] == group_n[h]:
                                epilogue(h)
    split_sync_waits(nc)
    return nc


def _quantize(query: np.ndarray, database: np.ndarray):
    """Row-normalize to norm SCALE and quantize to fp8 e4m3 (TRN)."""
    q = np.asarray(query, dtype=np.float32).reshape(1, D)
    db = np.asarray(database, dtype=np.float32)
    dn = np.sqrt(np.einsum("ij,ij->i", db, db))
    dn = np.maximum(dn, COS_EPS)
    qn = max(float(np.linalg.norm(q)), COS_EPS)
    db8 = (db * (SCALE / dn)[:, None]).astype(_np_f8)
    q8 = (q[0] * (SCALE / qn)).astype(_np_f8)
    return q8, db8


def _prep_inputs(query: np.ndarray, database: np.ndarray, n_cores: int = N_CORES,
                 shard: int = SHARD, paired: bool = True):
    q8, db8 = _quantize(query, database)

    qs = np.zeros((P, C, 256), dtype=_np_f8)
    for c in range(C):
        qs[:, c, P] = q8[c * P : (c + 1) * P]

    pad_row = (-q8.astype(np.float32)).astype(_np_f8)
    in_maps = []
    for cr in range(n_cores):
        sh = np.empty((PADROWS, D), dtype=_np_f8)
        sh[:shard] = db8[cr * shard : (cr + 1) * shard]
        sh[shard:] = pad_row
        dbt = np.ascontiguousarray(sh.T)          # [768, 62976] fp8
        if paired:
            # [3 pairs][128 dims][2 ko][rows]: one DMA per (block, pair)
            dbt = np.ascontiguousarray(
                dbt.reshape(C // 2, 2, P, PADROWS).transpose(0, 2, 1, 3))
        in_maps.append({"dbt": dbt.reshape(-1), "qs": qs})
    return in_maps


def _host_reduce(results, query: np.ndarray, database: np.ndarray,
                 n_cores: int = N_CORES, shard: int = SHARD) -> np.ndarray:
    q = np.asarray(query, dtype=np.float32).reshape(1, D)
    db = np.asarray(database, dtype=np.float32)

    cols = np.stack([r["outi"] for r in results]).astype(np.int64)  # [Cn,n_out,8]
    n_out = cols.shape[1]
    win_map = np.full(n_out, -1, dtype=np.int64)
    if n_out == P:            # windows = partitions directly
        win_map[:NW] = np.arange(NW)
    else:                     # split epilogue: two halves of 62 + 61 windows
        win_map[0:62] = np.arange(62)
        win_map[P : P + (NW - 62)] = 62 + np.arange(NW - 62)
    c_idx = np.arange(n_cores, dtype=np.int64)[:, None, None]
    w_idx = win_map[None, :, None]

    shard_row = w_idx * WIN + cols
    valid = (w_idx >= 0) & (shard_row < shard)
    gidx = (c_idx * shard + shard_row)[valid].ravel()
    gidx = np.unique(gidx)

    # exact rescore of the candidate set
    rows = db[gidx].astype(np.float64)
    qd = q[0].astype(np.float64)
    qn = max(float(np.linalg.norm(qd)), COS_EPS)
    rn = np.maximum(np.linalg.norm(rows, axis=1), COS_EPS)
    sims = (rows @ qd) / (rn * qn)

    top = np.argsort(-sims, kind="stable")[:K]
    s = sims[top]
    idx = gidx[top]

    d = 1.0 - s
    w = 1.0 / (d + W_EPS) ** 2
    w = w / w.sum()
    centroid = (w[None, :] @ db[idx].astype(np.float64)).astype(np.float32)
    return centroid  # [1, D]


def _run(query: np.ndarray, database: np.ndarray, trace: bool = False):
    nc = build_nc()
    in_maps = _prep_inputs(query, database)
    res = run_bass_kernel_spmd(
        nc, in_maps, core_ids=list(range(N_CORES)), trace=trace,
    )
    out = _host_reduce(res.results, query, database)
    return out, res


def kernel(query: np.ndarray, database: np.ndarray) -> np.ndarray:
    out, _ = _run(query, database, trace=False)
    return out
